# revision 22
# baseline (speedup 1.0000x reference)
"""Trainium2 Bass kernel for CapsNet dynamic routing (nn_Model_16492674417055).

Reference computation:
    u_hat[b,i,j,c,p] = sum_q w[j,c,p,q] x[b,i,c,q]
    3 routing iterations of: c = softmax_j(b); s = sum_i c*u_hat;
    v = squash(s); a = <u_hat, v>; b += a. Output v of last iteration.

Key algebraic factorization (exact in real arithmetic): u_hat never needs to
be materialized (it is 1 GiB).  With xc[b,j,c,:] = sum_i c[b,i,j,c] x[b,i,c,:]:
    s  = W @ xc
    a  = <x_i, W^T v>  and  W^T v = kappa * (W^T W) xc = kappa * G xc,
where kappa is the squash scale, computable from |s|^2 = <xc, G xc>.
So iterations 1..2 need only G = W^T W (host-precomputed), and the final
iteration needs one true W application for the output direction.

Precision: all matmul inputs are fp16 (x, G, wt, softmax weights c, xc, vt);
accumulation is fp32 in PSUM; logits/softmax/squash scalars are fp32.  fp16
(10 mantissa bits) keeps the sharp routing softmax accurate: measured 2.0e-3
relative error on the seed-0 inputs (vs 1.6e-2 for bf16, 3.7e-5 for fp32).
The xc*gx products reach ~6e5 > fp16 max, so xg stays fp32 (fp32 ones-matmul
for |s|^2). fp16 also quarters PE matmul cost vs fp32 and halves DMA.

Sharding: data-parallel over batch B=16 across 8 cores (2 batches/core);
G / wT are replicated (loaded per core); routing state stays core-local.
"""

import numpy as np

import concourse.bass as bass
import concourse.tile as tile
from concourse import bacc
from concourse import mybir
from concourse.alu_op_type import AluOpType as AO
from concourse.bass import MemorySpace
from concourse.bass_utils import run_bass_kernel_spmd
from concourse.masks import make_identity

F32 = mybir.dt.float32
F16 = mybir.dt.float16
AXX = mybir.AxisListType.X
AF = mybir.ActivationFunctionType

N_CORES = 8
B, N_PRE, N_DIGIT, CH, D = 16, 1024, 32, 4, 128
BL = B // N_CORES          # batches per core (2)
NCHUNK = N_PRE // 128      # i-chunks (8)
NJC = N_DIGIT * CH         # 128 (j,c) pairs
EPS = 1e-7
N_ITERS = 3


class _Bacc(bacc.Bacc):
    """Bacc whose ACT-table chooser only sees natural_log_exp_and_others, so
    alternating Exp (softmax) / Ln+Exp (squash sqrt) stay on ONE table set
    (one LoadActFuncSet instead of one per switch)."""

    def insert_act_table_loads(self):
        from concourse.hw_specs import get_activation_tables

        has_activation = any(
            isinstance(i, mybir.InstActivation)
            for b in self.main_func.blocks
            for i in b.instructions
        )
        if not has_activation:
            return
        tables = [
            (n, fns if n == "natural_log_exp_and_others" else set())
            for n, fns in get_activation_tables(self.m.arch).items()
        ]
        bacc._bass_rust.insert_act_table_loads(self, tables)


def build_nc(bench_reps: int = 0, bench_mode: str = "full") -> bass.Bass:
    """bench_reps>0 wraps the whole kernel body (input DMAs included) in a
    For_i loop of that many reps inside one NEFF, for wall-clock timing that
    amortizes the multi-ms axon dispatch floor."""
    nc = _Bacc()

    # Per-core DRAM inputs, host pre-laid-out so every load is a straight
    # [128, N] partition-major copy.  All fp16.
    xk_d = nc.declare_dram_parameter("xk", [128, BL, CH, NCHUNK, 128], F16, isOutput=False)  # [i128, b, c, k, q]
    xt_d = nc.declare_dram_parameter("xt", [128, BL, CH, NCHUNK, 128], F16, isOutput=False)  # [q, b, c, k, i128]
    g_d = nc.declare_dram_parameter("g", [128, NJC, 128], F16, isOutput=False)               # [r, (c j), q]
    wt_d = nc.declare_dram_parameter("wt", [128, NJC, 128], F16, isOutput=False)             # [q, (c j), p]
    out_d = nc.declare_dram_parameter("out", [BL, N_DIGIT, CH, D], F32, isOutput=True)

    with tile.TileContext(nc) as tc:
        with (
            tc.tile_pool(name="big", bufs=1) as big,
            tc.tile_pool(name="sm", bufs=2) as sm,
            tc.tile_pool(name="ps_mix", bufs=2, space=MemorySpace.PSUM) as ps_mix,
            tc.tile_pool(name="ps_gk", bufs=2, space=MemorySpace.PSUM) as ps_gk,
        ):
            # ---- static tiles ----
            xk = big.tile([128, BL, CH, NCHUNK, 128], F16, tag="xk")
            xt = big.tile([128, BL, CH, NCHUNK, 128], F16, tag="xt")
            g_t = big.tile([128, NJC, 128], F16, tag="g")
            wt_t = big.tile([128, NJC, 128], F16, tag="wt")

            c_unif = big.tile([128, N_DIGIT], F16, tag="c_unif")
            nc.vector.memset(c_unif, 1.0 / N_DIGIT)
            ones_col = big.tile([128, 1], F32, tag="ones_col")
            nc.vector.memset(ones_col, 1.0)
            ones_row = big.tile([1, 128], F16, tag="ones_row")
            nc.vector.memset(ones_row, 1.0)
            ident = big.tile([128, 128], F32, tag="ident")
            make_identity(nc, ident[:])
            eps_t = big.tile([1, 1], F32, tag="eps_t")
            nc.vector.memset(eps_t, EPS)

            # routing logits, both local batches: [i%128, b, k, c, j]  fp32
            bl_t = big.tile([128, BL, NCHUNK, CH, N_DIGIT], F32, tag="bl")

            def trace_loads():
                for b in range(BL):
                    nc.sync.dma_start(out=xk[:, b], in_=xk_d[:, b])
                for b in range(BL):
                    nc.sync.dma_start(out=xt[:, b], in_=xt_d[:, b])
                for gc in range(4):
                    nc.scalar.dma_start(
                        out=g_t[:, gc * 32 : (gc + 1) * 32, :],
                        in_=g_d[:, gc * 32 : (gc + 1) * 32, :],
                    )
                for gc in range(4):
                    nc.scalar.dma_start(
                        out=wt_t[:, gc * 32 : (gc + 1) * 32, :],
                        in_=wt_d[:, gc * 32 : (gc + 1) * 32, :],
                    )

            def trace_body(loads=True, compute=True):
              if loads:
                trace_loads()
              if not compute:
                return
              for t in range(N_ITERS):
                  last = t == N_ITERS - 1

                  # ---- softmax over j (t=0: uniform, skip) ----
                  # two independent per-batch chains so XC(b=0) starts while
                  # b=1's chain finishes.  max-subtract fp32 (DVE b0 / Pool
                  # b1); exp output fp16 (args <=0) so the rest of the chain
                  # runs in DVE 2x mode.
                  cbt = []
                  if t > 0:
                      mx = sm.tile([128, BL, NCHUNK, CH], F32, tag="mx")
                      for b in range(BL):
                          eb = sm.tile([128, NCHUNK, CH, N_DIGIT], F32, tag=f"eb{b}")
                          e16 = sm.tile([128, NCHUNK, CH, N_DIGIT], F16, tag=f"e16{b}")
                          sb = sm.tile([128, NCHUNK, CH], F16, tag=f"sum{b}")
                          cb = sm.tile([128, NCHUNK, CH, N_DIGIT], F16, tag=f"cb{b}")
                          ve = nc.vector if b == 0 else nc.gpsimd
                          nc.vector.reduce_max(out=mx[:, b], in_=bl_t[:, b], axis=AXX, negate=True)
                          ve.tensor_add(eb[:], bl_t[:, b], mx[:, b].to_broadcast(eb.shape))
                          nc.scalar.activation(e16[:], eb[:], AF.Exp)
                          with nc.allow_low_precision(reason="softmax weights only need ~0.1%; fp16 keeps DVE in 2x mode"):
                              nc.vector.reduce_sum(out=sb[:], in_=e16[:], axis=AXX)
                              nc.vector.reciprocal(sb[:], sb[:])
                          nc.vector.tensor_mul(cb[:], e16[:], sb[:].to_broadcast(e16.shape))
                          cbt.append(cb)

                  # ---- XC: xcT[q, (j b)] per c ----
                  xc_sb = [sm.tile([128, N_DIGIT, BL], F16, tag=f"xc{c}", name=f"xc{c}", bufs=3) for c in range(CH)]
                  for c in range(CH):
                      for b in range(BL):
                          xc_ps = ps_mix.tile([128, N_DIGIT], F32, tag="xc_ps")
                          for k in range(NCHUNK):
                              rhs = cbt[b][:, k, c, :] if t > 0 else c_unif[:]
                              nc.tensor.matmul(
                                  xc_ps[:],
                                  lhsT=xk[:, b, c, k, :],
                                  rhs=rhs,
                                  start=(k == 0),
                                  stop=(k == NCHUNK - 1),
                              )
                          if b == 0:
                              nc.vector.tensor_copy(xc_sb[c][:, :, b], xc_ps[:])
                          else:
                              nc.scalar.copy(out=xc_sb[c][:, :, b], in_=xc_ps[:])

                  # ---- W-pass: gxT = G @ xc (t<2)  /  sT = W @ xc (t=2) ----
                  wsrc = wt_t if last else g_t
                  gx_ps = ps_gk.tile([128, CH, N_DIGIT, BL], F32, tag="gk")
                  for jc in range(NJC):
                      c, j = divmod(jc, N_DIGIT)
                      nc.tensor.matmul(
                          gx_ps[:, c, j, :],
                          lhsT=wsrc[:, jc, :],
                          rhs=xc_sb[c][:, j, :],
                          start=True,
                          stop=True,
                      )

                  # ---- squash scale kappa; vt = kappa*gx (whole-tile) ----
                  gx_sb = sm.tile([128, CH, N_DIGIT, BL], F16, tag="gx_sb", bufs=3)
                  nc.scalar.copy(out=gx_sb[:], in_=gx_ps[:])
                  xg = sm.tile([128, CH, N_DIGIT, BL], F32, tag="xg")
                  # t<2: |s|^2 = <xc, G xc>;  t=2: |s|^2 = <s, s>
                  if not last:
                      for c in range(CH):
                          nc.vector.tensor_mul(xg[:, c], xc_sb[c][:], gx_sb[:, c])
                  else:
                      nc.vector.tensor_mul(xg[:], gx_sb[:], gx_sb[:])
                  # sq lives in row 0 of the kb bank (saves a PSUM bank); the
                  # later kb matmul overwrites it only after kap is computed.
                  kb_ps = ps_gk.tile([128, CH, N_DIGIT, BL], F32, tag="gk")
                  sq_ps = kb_ps[0:1].rearrange("p a b c -> p (a b c)")
                  nc.tensor.matmul(
                      sq_ps,
                      lhsT=ones_col[:],
                      rhs=xg[:].rearrange("p a b c -> p (a b c)"),
                      start=True,
                      stop=True,
                  )
                  # kappa = sq/((1+sq)*sqrt(sq+eps)); sqrt = exp(0.5*ln) so
                  # only the natural_log_exp ACT table set is used.
                  t1 = sm.tile([1, CH * N_DIGIT * BL], F32, tag="t1")
                  t2 = sm.tile([1, CH * N_DIGIT * BL], F32, tag="t2")
                  kap = sm.tile([1, CH * N_DIGIT * BL], F16, tag="kap")
                  nc.scalar.activation(t1[:], sq_ps, AF.Ln, bias=eps_t[:])
                  nc.scalar.activation(t1[:], t1[:], AF.Exp, scale=0.5)
                  nc.vector.scalar_tensor_tensor(
                      out=t2[:], in0=sq_ps, scalar=1.0,
                      in1=t1[:], op0=AO.add, op1=AO.mult,
                  )
                  nc.vector.reciprocal(t2[:], t2[:])
                  nc.vector.tensor_mul(kap[:], sq_ps, t2[:])
                  nc.tensor.matmul(
                      kb_ps[:].rearrange("p a b c -> p (a b c)"),
                      lhsT=ones_row[:],
                      rhs=kap[:],
                      start=True,
                      stop=True,
                  )
                  if not last:
                      # kappa stays out of the A-pass: a = kappa*(x.gx) is
                      # applied at the logits update, so the A-pass matmuls
                      # (PE) run on raw gx_sb concurrently with this kappa
                      # chain.  kb goes to SBUF fp16 for that update.
                      kb_sb = sm.tile([128, CH, N_DIGIT, BL], F16, tag="kb_sb", bufs=3)
                      nc.scalar.copy(out=kb_sb[:], in_=kb_ps[:])
                  else:
                      vt32 = sm.tile([128, CH, N_DIGIT, BL], F32, tag="vt32")
                      nc.vector.tensor_mul(vt32[:], gx_sb[:], kb_ps[:])

                  if not last:
                      # ---- A-pass: araw[i,(c j)] = sum_q x[i,q] gx[j,q];
                      # bl (+)= kappa*araw.  4 k-chunks share one full PSUM
                      # bank so the logits update is 4/8 big DVE ops.
                      for b in range(BL):
                          for kh in range(2):
                              a_ps = ps_mix.tile([128, 4, CH, N_DIGIT], F32, tag="a")
                              for kk in range(4):
                                  k = kh * 4 + kk
                                  for c in range(CH):
                                      nc.tensor.matmul(
                                          a_ps[:, kk, c, :],
                                          lhsT=xt[:, b, c, k, :],
                                          rhs=gx_sb[:, c, :, b],
                                          start=True,
                                          stop=True,
                                      )
                              dst = bl_t[:, b, kh * 4 : kh * 4 + 4]
                              kbb = kb_sb[:, :, :, b].rearrange(
                                  "p (o c) j -> p o c j", o=1
                              ).to_broadcast([128, 4, CH, N_DIGIT])
                              if t == 0:
                                  nc.vector.scalar_tensor_tensor(
                                      out=dst, in0=a_ps[:], scalar=1.0,
                                      in1=kbb, op0=AO.bypass, op1=AO.mult,
                                  )
                              else:
                                  at = sm.tile([128, 4, CH, N_DIGIT], F32, tag="at")
                                  nc.vector.scalar_tensor_tensor(
                                      out=at[:], in0=a_ps[:], scalar=1.0,
                                      in1=kbb, op0=AO.bypass, op1=AO.mult,
                                  )
                                  nc.vector.tensor_add(dst, dst, at[:])
                  else:
                      # ---- output: transpose v [p, (c,j,b)] -> [(c,j,b), p], DMA ----
                      vflat = vt32[:].rearrange("p a b c -> p (a b c)")
                      out_ap = out_d[:].rearrange("b j c p -> c j b p")  # [4,32,2,128]
                      tr_t = ps_gk.tile([128, CH, N_DIGIT, BL], F32, tag="gk")
                      trv = tr_t[:].rearrange("p a b c -> p (a b c)")
                      for half in range(2):
                          nc.tensor.transpose(
                              trv[:, half * 128 : (half + 1) * 128],
                              vflat[:, half * 128 : (half + 1) * 128], ident[:]
                          )
                      ob = sm.tile([128, 2, 128], F32, tag="ob")
                      nc.vector.tensor_copy(ob[:].rearrange("p a b -> p (a b)"), trv)
                      for half in range(2):
                          for cl in range(2):
                              nc.sync.dma_start(
                                  out=out_ap[half * 2 + cl],
                                  in_=ob[cl * 64 : (cl + 1) * 64, half, :],
                              )

            if bench_reps:
                if bench_mode == "nodma":
                    trace_loads()
                with tc.For_i(0, bench_reps, 1):
                    trace_body(loads=(bench_mode != "nodma"),
                               compute=(bench_mode != "dmaonly"))
            else:
                trace_body()
    return nc


def _host_prep(x: np.ndarray, w: np.ndarray):
    """Host-side layout prep shared by all cores (w-derived) and per-core (x)."""
    x = np.ascontiguousarray(x, dtype=np.float32)
    w = np.ascontiguousarray(w, dtype=np.float32)
    # G[j,c,q,r] = sum_p w[j,c,p,q] w[j,c,p,r]
    wf = np.ascontiguousarray(w.transpose(1, 0, 2, 3)).reshape(NJC, D, D)  # jc = c*32+j
    G = np.matmul(wf.transpose(0, 2, 1), wf)                 # [jc, q, r]
    g_h = np.ascontiguousarray(G.transpose(1, 0, 2)).astype(np.float16)    # [q, jc, r]
    wt_h = np.ascontiguousarray(wf.transpose(2, 0, 1)).astype(np.float16)  # [q, jc, p]
    # x[b,i,c,q] with i = k*128 + r  ->  xk [r, b, c, k, q], xt [q, b, c, k, r]
    xr = x.reshape(B, NCHUNK, 128, CH, D)
    xk_h = np.ascontiguousarray(xr.transpose(2, 0, 3, 1, 4)).astype(np.float16)  # [r, b, c, k, q]
    xt_h = np.ascontiguousarray(xr.transpose(4, 0, 3, 1, 2)).astype(np.float16)  # [q, b, c, k, r]
    return xk_h, xt_h, g_h, wt_h


def _run(x: np.ndarray, w: np.ndarray, **spmd_kwargs):
    xk_h, xt_h, g_h, wt_h = _host_prep(x, w)
    in_maps = []
    for core in range(N_CORES):
        in_maps.append(
            {
                "xk": xk_h[:, core * BL : (core + 1) * BL],
                "xt": xt_h[:, core * BL : (core + 1) * BL],
                "g": g_h,
                "wt": wt_h,
            }
        )
    nc = build_nc()
    nc.finalize()
    res = run_bass_kernel_spmd(nc, in_maps, list(range(N_CORES)), **spmd_kwargs)
    out = np.concatenate([res.results[c]["out"] for c in range(N_CORES)], axis=0)
    return out.astype(np.float32), res


def kernel(x: np.ndarray, w: np.ndarray) -> np.ndarray:
    out, _ = _run(x, w)
    return out


# revision 25
# speedup vs baseline: 1.1110x; 1.1110x over previous
"""Trainium2 Bass kernel for CapsNet dynamic routing (nn_Model_16492674417055).

Reference computation:
    u_hat[b,i,j,c,p] = sum_q w[j,c,p,q] x[b,i,c,q]
    3 routing iterations of: c = softmax_j(b); s = sum_i c*u_hat;
    v = squash(s); a = <u_hat, v>; b += a. Output v of last iteration.

Key algebraic factorization (exact in real arithmetic): u_hat never needs to
be materialized (it is 1 GiB).  With xc[b,j,c,:] = sum_i c[b,i,j,c] x[b,i,c,:]:
    s  = W @ xc
    a  = <x_i, W^T v>  and  W^T v = kappa * (W^T W) xc = kappa * G xc,
where kappa is the squash scale, computable from |s|^2 = <xc, G xc>.
So iterations 1..2 need only G = W^T W (host-precomputed), and the final
iteration needs one true W application for the output direction.

Sharding: the routing is fully independent per channel ch (softmax couples
only the n_digit axis), so the 16 batches x 4 channels factor into 64
independent problems.  Each of the 8 cores takes 8 batches x 1 channel
(core k: ch=k//2, batch half k%2).  vs. pure batch sharding this makes the
per-(j,ch) G-matvecs 8 columns wide (32 matmuls/iter instead of 128 - the
PE is weight-load bound so narrow matmuls waste it) and loads only the
ch-slice of G/wT per core (6 MiB total DMA instead of 12).

Precision: all matmul inputs fp16 (10 mantissa bits; measured ~2.5e-3 final
relative error vs 1.6e-2 for bf16 which breaks the sharp routing softmax),
accumulation fp32 in PSUM, logits fp32, squash scalars fp32.  The xc*gx
products reach ~6e5 > fp16 max so the |s|^2 pieces stay fp32.  kappa is
applied at the logits update (a = kappa*(x.gx)) so the A-pass matmuls run
on raw gx concurrently with the kappa chain.
"""

import numpy as np

import concourse.bass as bass
import concourse.tile as tile
from concourse import bacc
from concourse import mybir
from concourse.alu_op_type import AluOpType as AO
from concourse.bass import MemorySpace
from concourse.bass_utils import run_bass_kernel_spmd
from concourse.masks import make_identity

F32 = mybir.dt.float32
F16 = mybir.dt.float16
AXX = mybir.AxisListType.X
AF = mybir.ActivationFunctionType

N_CORES = 8
B, N_PRE, N_DIGIT, CH, D = 16, 1024, 32, 4, 128
BLC = 8                    # batches per core (half of B)
NCHUNK = N_PRE // 128      # i-chunks (8)
EPS = 1e-7
N_ITERS = 3
NJB = N_DIGIT * BLC        # 256 (j,b) pairs per core


class _Bacc(bacc.Bacc):
    """Bacc whose ACT-table chooser only sees natural_log_exp_and_others, so
    alternating Exp (softmax) / Ln+Exp (squash sqrt) stay on ONE table set
    (one LoadActFuncSet instead of one per switch)."""

    def insert_act_table_loads(self):
        from concourse.hw_specs import get_activation_tables

        has_activation = any(
            isinstance(i, mybir.InstActivation)
            for b in self.main_func.blocks
            for i in b.instructions
        )
        if not has_activation:
            return
        tables = [
            (n, fns if n == "natural_log_exp_and_others" else set())
            for n, fns in get_activation_tables(self.m.arch).items()
        ]
        bacc._bass_rust.insert_act_table_loads(self, tables)


def build_nc(bench_reps: int = 0, bench_mode: str = "full") -> bass.Bass:
    """bench_reps>0 wraps the whole kernel body (input DMAs included) in a
    For_i loop of that many reps inside one NEFF, for wall-clock timing that
    amortizes the multi-ms axon dispatch floor."""
    nc = _Bacc()

    # Per-core DRAM inputs, host pre-laid-out so every load is a straight
    # [128, N] partition-major copy.  All fp16; single channel per core.
    xk_d = nc.declare_dram_parameter("xk", [128, BLC, NCHUNK, 128], F16, isOutput=False)  # [i128, b, k, q]
    xt_d = nc.declare_dram_parameter("xt", [128, BLC, NCHUNK, 128], F16, isOutput=False)  # [q, b, k, i128]
    g_d = nc.declare_dram_parameter("g", [128, N_DIGIT, 128], F16, isOutput=False)        # [r, j, q]
    wt_d = nc.declare_dram_parameter("wt", [128, N_DIGIT, 128], F16, isOutput=False)      # [q, j, p]
    out_d = nc.declare_dram_parameter("out", [BLC, N_DIGIT, D], F32, isOutput=True)

    with tile.TileContext(nc) as tc:
        with (
            tc.tile_pool(name="big", bufs=1) as big,
            tc.tile_pool(name="sm", bufs=2) as sm,
            tc.tile_pool(name="ps_xc", bufs=2, space=MemorySpace.PSUM) as ps_xc,
            tc.tile_pool(name="ps_gk", bufs=2, space=MemorySpace.PSUM) as ps_gk,
            tc.tile_pool(name="ps_a", bufs=2, space=MemorySpace.PSUM) as ps_a,
        ):
            # ---- static tiles ----
            xk = big.tile([128, BLC, NCHUNK, 128], F16, tag="xk")
            xt = big.tile([128, BLC, NCHUNK, 128], F16, tag="xt")
            g_t = big.tile([128, N_DIGIT, 128], F16, tag="g")
            wt_t = big.tile([128, N_DIGIT, 128], F16, tag="wt")

            c_unif = big.tile([128, N_DIGIT], F16, tag="c_unif")
            nc.vector.memset(c_unif, 1.0 / N_DIGIT)
            ones_col = big.tile([128, 1], F32, tag="ones_col")
            nc.vector.memset(ones_col, 1.0)
            ones_row = big.tile([1, 128], F16, tag="ones_row")
            nc.vector.memset(ones_row, 1.0)
            ident = big.tile([128, 128], F32, tag="ident")
            make_identity(nc, ident[:])
            eps_t = big.tile([1, 1], F32, tag="eps_t")
            nc.vector.memset(eps_t, EPS)

            # routing logits: [i%128, bpair, b%2, k, j]  fp32 (8 KiB/part)
            bl_t = big.tile([128, 4, 2, NCHUNK, N_DIGIT], F32, tag="bl")

            def trace_loads():
                for h in range(2):
                    nc.sync.dma_start(out=xk[:, h * 4 : h * 4 + 4], in_=xk_d[:, h * 4 : h * 4 + 4])
                for h in range(2):
                    nc.sync.dma_start(out=xt[:, h * 4 : h * 4 + 4], in_=xt_d[:, h * 4 : h * 4 + 4])
                nc.scalar.dma_start(out=g_t[:], in_=g_d[:])
                nc.scalar.dma_start(out=wt_t[:], in_=wt_d[:])

            def trace_body(loads=True, compute=True):
              if loads:
                trace_loads()
              if not compute:
                return
              for t in range(N_ITERS):
                  last = t == N_ITERS - 1

                  # ---- softmax over j (t=0: uniform, skip) ----
                  # fp32 max-subtract (DVE half / GpSimd half in parallel);
                  # exp output fp16 (args <=0) so the tail runs in DVE 2x mode.
                  cb = None
                  if t > 0:
                      mx = sm.tile([128, 4, 2, NCHUNK], F32, tag="mx")
                      eb = sm.tile([128, 4, 2, NCHUNK, N_DIGIT], F32, tag="eb")
                      e16 = sm.tile([128, 4, 2, NCHUNK, N_DIGIT], F16, tag="e16")
                      sb = sm.tile([128, 4, 2, NCHUNK], F16, tag="sum")
                      cb = sm.tile([128, 4, 2, NCHUNK, N_DIGIT], F16, tag="cb")
                      nc.vector.reduce_max(out=mx[:], in_=bl_t[:], axis=AXX, negate=True)
                      nc.vector.tensor_add(eb[:, 0:2], bl_t[:, 0:2], mx[:, 0:2].to_broadcast(eb[:, 0:2].shape))
                      nc.gpsimd.tensor_add(eb[:, 2:4], bl_t[:, 2:4], mx[:, 2:4].to_broadcast(eb[:, 2:4].shape))
                      nc.scalar.activation(e16[:], eb[:], AF.Exp)
                      with nc.allow_low_precision(reason="softmax weights only need ~0.1%; fp16 keeps DVE in 2x mode"):
                          nc.vector.reduce_sum(out=sb[:], in_=e16[:], axis=AXX)
                          nc.vector.reciprocal(sb[:], sb[:])
                      nc.vector.tensor_mul(cb[:], e16[:], sb[:].to_broadcast(e16.shape))

                  # ---- XC: xcT[q, j] per b -> xc_sb [q, j, b] ----
                  xc_sb = sm.tile([128, N_DIGIT, BLC], F16, tag="xc_sb", bufs=3)
                  for b in range(BLC):
                      xc_ps = ps_xc.tile([128, N_DIGIT], F32, tag="xc_ps")
                      for k in range(NCHUNK):
                          rhs = cb[:, b // 2, b % 2, k, :] if t > 0 else c_unif[:]
                          nc.tensor.matmul(
                              xc_ps[:],
                              lhsT=xk[:, b, k, :],
                              rhs=rhs,
                              start=(k == 0),
                              stop=(k == NCHUNK - 1),
                          )
                      if b % 2 == 0:
                          nc.vector.tensor_copy(xc_sb[:, :, b], xc_ps[:])
                      else:
                          nc.scalar.copy(out=xc_sb[:, :, b], in_=xc_ps[:])

                  # ---- W-pass: gxT[q, (j b)] = G_j @ xc (t<2) / W_j (t=2) ----
                  # one matmul per j with all 8 batches as the moving dim.
                  wsrc = wt_t if last else g_t
                  gx_ps = ps_gk.tile([128, N_DIGIT, BLC], F32, tag="gk")
                  for j in range(N_DIGIT):
                      nc.tensor.matmul(
                          gx_ps[:, j, :],
                          lhsT=wsrc[:, j, :],
                          rhs=xc_sb[:, j, :],
                          start=True,
                          stop=True,
                      )
                  gx_sb = sm.tile([128, N_DIGIT, BLC], F16, tag="gx_sb", bufs=3)
                  nc.scalar.copy(out=gx_sb[:], in_=gx_ps[:])

                  # ---- |s|^2 and kappa = sq/((1+sq)*sqrt(sq+eps)) ----
                  xg = sm.tile([128, N_DIGIT, BLC], F32, tag="xg")
                  if not last:
                      nc.vector.tensor_mul(xg[:], xc_sb[:], gx_sb[:])
                  else:
                      nc.vector.tensor_mul(xg[:], gx_sb[:], gx_sb[:])
                  # sq lives in row 0 of the kb tile's bank (saves a bank)
                  kb_ps = ps_gk.tile([128, N_DIGIT, BLC], F32, tag="gk")
                  sq_ps = kb_ps[0:1].rearrange("p a b -> p (a b)")
                  nc.tensor.matmul(
                      sq_ps,
                      lhsT=ones_col[:],
                      rhs=xg[:].rearrange("p a b -> p (a b)"),
                      start=True,
                      stop=True,
                  )
                  t1 = sm.tile([1, NJB], F32, tag="t1")
                  t2 = sm.tile([1, NJB], F32, tag="t2")
                  kap = sm.tile([1, NJB], F16, tag="kap")
                  # sqrt = exp(0.5*ln) keeps everything on one ACT table set
                  nc.scalar.activation(t1[:], sq_ps, AF.Ln, bias=eps_t[:])
                  nc.scalar.activation(t1[:], t1[:], AF.Exp, scale=0.5)
                  nc.vector.scalar_tensor_tensor(
                      out=t2[:], in0=sq_ps, scalar=1.0,
                      in1=t1[:], op0=AO.add, op1=AO.mult,
                  )
                  nc.vector.reciprocal(t2[:], t2[:])
                  nc.vector.tensor_mul(kap[:], sq_ps, t2[:])
                  nc.tensor.matmul(
                      kb_ps[:].rearrange("p a b -> p (a b)"),
                      lhsT=ones_row[:],
                      rhs=kap[:],
                      start=True,
                      stop=True,
                  )

                  if not last:
                      # kappa applies at the logits update so the A-pass runs
                      # on raw gx_sb in parallel with the kappa chain above.
                      kb_sb = sm.tile([128, N_DIGIT, BLC], F16, tag="kb_sb", bufs=3)
                      nc.scalar.copy(out=kb_sb[:], in_=kb_ps[:])
                      # ---- A-pass: araw[i,(k j)] = sum_q x[i,q] gx[j,q];
                      # bl (+)= kappa*araw, one/two DVE ops per batch pair ----
                      for bp in range(4):
                          a_ps = ps_a.tile([128, 2, NCHUNK, N_DIGIT], F32, tag="a")
                          for bb in range(2):
                              b = bp * 2 + bb
                              for k in range(NCHUNK):
                                  nc.tensor.matmul(
                                      a_ps[:, bb, k, :],
                                      lhsT=xt[:, b, k, :],
                                      rhs=gx_sb[:, :, b],
                                      start=True,
                                      stop=True,
                                  )
                          at = None
                          if t > 0:
                              at = sm.tile([128, 2, NCHUNK, N_DIGIT], F32, tag="at")
                          for bb in range(2):
                              b = bp * 2 + bb
                              # STT is limited to 3D operands, so per-batch
                              kbb = (
                                  kb_sb[:, :, b]
                                  .rearrange("p (o j) -> p o j", o=1)
                                  .to_broadcast([128, NCHUNK, N_DIGIT])
                              )
                              o3 = bl_t[:, bp, bb] if t == 0 else at[:, bb]
                              nc.vector.scalar_tensor_tensor(
                                  out=o3, in0=a_ps[:, bb], scalar=1.0,
                                  in1=kbb, op0=AO.bypass, op1=AO.mult,
                              )
                          if t > 0:
                              nc.vector.tensor_add(bl_t[:, bp], bl_t[:, bp], at[:])
                  else:
                      # ---- output: v = kappa*s; transpose [p,(j b)] ->
                      # [(j b), p]; DMA out ----
                      vt32 = sm.tile([128, N_DIGIT, BLC], F32, tag="vt32")
                      nc.vector.tensor_mul(vt32[:], gx_sb[:], kb_ps[:])
                      vflat = vt32[:].rearrange("p a b -> p (a b)")
                      out_ap = out_d[:].rearrange("b j p -> j b p")  # [32, 8, 128]
                      tr_t = ps_gk.tile([128, N_DIGIT, BLC], F32, tag="gk")
                      trv = tr_t[:].rearrange("p a b -> p (a b)")
                      for half in range(2):
                          nc.tensor.transpose(
                              trv[:, half * 128 : (half + 1) * 128],
                              vflat[:, half * 128 : (half + 1) * 128], ident[:]
                          )
                      ob = sm.tile([128, 2, 128], F32, tag="ob")
                      nc.vector.tensor_copy(ob[:].rearrange("p a b -> p (a b)"), trv)
                      # ob[:, half, :] rows are the (j,b) pairs 128*half..:
                      # row r = (j, b) = divmod(128*half + r, 8)
                      for half in range(2):
                          nc.sync.dma_start(
                              out=out_ap[half * 16 : (half + 1) * 16],
                              in_=ob[:, half, :],
                          )

            if bench_reps:
                if bench_mode == "nodma":
                    trace_loads()
                with tc.For_i(0, bench_reps, 1):
                    trace_body(loads=(bench_mode != "nodma"),
                               compute=(bench_mode != "dmaonly"))
            else:
                trace_body()
    return nc


def _host_prep(x: np.ndarray, w: np.ndarray):
    """Host-side layout prep: per-channel W-derived tensors + x layouts."""
    x = np.ascontiguousarray(x, dtype=np.float32)
    w = np.ascontiguousarray(w, dtype=np.float32)
    # G[c,j,q,r] = sum_p w[j,c,p,q] w[j,c,p,r]
    wf = np.ascontiguousarray(w.transpose(1, 0, 2, 3))      # [c, j, p, q]
    G = np.matmul(wf.transpose(0, 1, 3, 2), wf)             # [c, j, q, r]
    g_h = np.ascontiguousarray(G.transpose(0, 2, 1, 3)).astype(np.float16)    # [c, q, j, r]
    wt_h = np.ascontiguousarray(wf.transpose(0, 3, 1, 2)).astype(np.float16)  # [c, q, j, p]
    # x[b,i,c,q] with i = k*128 + r  ->  xk [c, r, b, k, q], xt [c, q, b, k, r]
    xr = x.reshape(B, NCHUNK, 128, CH, D)
    xk_h = np.ascontiguousarray(xr.transpose(3, 2, 0, 1, 4)).astype(np.float16)  # [c, r, b, k, q]
    xt_h = np.ascontiguousarray(xr.transpose(3, 4, 0, 1, 2)).astype(np.float16)  # [c, q, b, k, r]
    return xk_h, xt_h, g_h, wt_h


def make_in_maps(x: np.ndarray, w: np.ndarray):
    """Per-core input dict: core k -> channel k//2, batch half k%2."""
    xk_h, xt_h, g_h, wt_h = _host_prep(x, w)
    in_maps = []
    for core in range(N_CORES):
        c, h = divmod(core, 2)
        bs = slice(h * BLC, (h + 1) * BLC)
        in_maps.append(
            {
                "xk": xk_h[c][:, bs],
                "xt": xt_h[c][:, bs],
                "g": g_h[c],
                "wt": wt_h[c],
            }
        )
    return in_maps


def _run(x: np.ndarray, w: np.ndarray, **spmd_kwargs):
    in_maps = make_in_maps(x, w)
    nc = build_nc()
    nc.finalize()
    res = run_bass_kernel_spmd(nc, in_maps, list(range(N_CORES)), **spmd_kwargs)
    # core k holds v[batch half k%2, :, ch k//2, :] as [BLC, N_DIGIT, D]
    out = np.empty((B, N_DIGIT, CH, D), dtype=np.float32)
    for core in range(N_CORES):
        c, h = divmod(core, 2)
        out[h * BLC : (h + 1) * BLC, :, c, :] = res.results[core]["out"]
    return out, res


def kernel(x: np.ndarray, w: np.ndarray) -> np.ndarray:
    out, _ = _run(x, w)
    return out
